# revision 45
# baseline (speedup 1.0000x reference)
"""Multi-head causal attention (B=2, L=2048, D=1024, H=16) on 8 trn2 cores.

Sharding: data-parallel over batch (2) x tensor-parallel over heads (4 groups
of 4 heads).  Core c handles batch c//4, heads 4*(c%4) .. 4*(c%4)+3.
Wq/Wk/Wv are column-sharded, Wo row-sharded; the TP all-reduce after Wo is
done host-side (sum of the 4 partial outputs per batch), as is the bo add.

Per-core kernel. Matmul precision scheme (all accumulation fp32 in PSUM):
  - Q/K/V projections run in fp8e4m3 with the DoubleRow perf mode (two
    128-row contraction chunks per instruction at 0.5 cycles/row).  To keep
    bf16-grade accuracy, X and W are split host-side into hi+lo fp8 pairs
    (w quantized after a x32 pre-scale so the residual stays in fp8's normal
    range; the 1/32 is folded into the PSUM->SBUF copy).  Each chunk-pair
    needs 3 DoubleRow matmuls (hi*hi, hi*lo, lo*hi; the lo*lo term is ~0.1%
    and dropped), so a projection costs 12 instrs at 0.5 cycles/row vs the
    bf16 16... i.e. 0.75x the bf16 PE time.
  - S^T = K^T Q runs in plain fp8 (Q/K quantized once at the projection
    copy; adds ~1.2e-2 rel err, the dominant error term).  The dk=64
    contraction is split 32+32 across the DoubleRow halves: q_s/k_s hold
    [partition = head*32 + dk%32, half = dk//32, q] (host permutes Wq/Wk
    columns so the projection lands directly in this layout), making each
    S block one DoubleRow matmul per head at 0.5 cycles/row - 2x the bf16
    tensor time.
  - P = exp(S/8) stays bf16 on the ACT engine (softmax WITHOUT
    max-subtraction; scores are bounded ~4 so exp is safe); PV and Wo
    matmuls stay bf16 (P has no cheap fp8 residual path and plain-fp8
    P/V/o would each cost ~2.5e-2 rel err).
  - a ones-row appended to V (lhsT [128, 65]) makes the PV matmul also emit
    softmax denominators; DVE reciprocal + gpsimd partition broadcast + one
    DVE multiply normalize O^T before the Wo matmul.
  - causal mask: S^T/exp/PV restricted to q >= k-tile start; the diagonal
    128x128 block is masked by one [128,2,128] gpsimd multiply with a
    broadcast upper-triangular 0/1 tile (supplied as input).
  - scheduling: inside each attention unit the S/exp/mask stream is emitted
    at high priority and the PV/normalize stream at normal priority, so a
    PV psum-pool wait can never block the next unit's S/exp on the in-order
    PE queue (pt depth 22 buffers the lag); o_ps gets its own 2-buffer psum
    tag so attention psum rotation is decoupled from the proj/wo tiles;
    wq/wk and the strip-0 x columns are host-packed into arena chunks so
    the first projection matmuls start after one small DMA; later x strips
    are prefetched up-front, partial outputs stored fp16, host upcasts.
"""

import numpy as np

B, L, D, H = 2, 2048, 1024, 16
DK = D // H          # 64
NCORES = 8
TP = 4               # head-group shards per batch
HG = H // TP         # 4 heads per core
DH = HG * DK         # 256 per-core head dims
STRIP = 512          # attention q-strip width
NSTRIP = L // STRIP  # 4
SUB = 512            # projection substrip width
NSUB = L // SUB      # 4
KT = 128             # key tile
NKT = L // KT        # 16
WS = 32.0            # host pre-scale on Wq/Wk/Wv so fp8 residuals stay normal

_CACHE = {}


def _build(causal: bool, qkv_bias: bool = True):
    import concourse.mybir as mybir
    import concourse.tile as tile
    from concourse import bacc

    f32 = mybir.dt.float32
    f32r = mybir.dt.float32r
    bf16 = mybir.dt.bfloat16
    f8 = mybir.dt.float8e4
    EXP = mybir.ActivationFunctionType.Exp
    DR = mybir.MatmulPerfMode.DoubleRow

    nc = bacc.Bacc("TRN2", target_bir_lowering=False)

    # x inputs as [128, chunk(8), hi/lo(2), L] fp8 (partition-major so a
    # strip slice is a 3-dim DMA: (chunk, hi/lo) merge)
    qT = nc.dram_tensor("qT", [128, 8, 2, L], f8, kind="ExternalInput")
    kT = nc.dram_tensor("kT", [128, 8, 2, L], f8, kind="ExternalInput")
    vT = nc.dram_tensor("vT", [128, 8, 2, L], f8, kind="ExternalInput")
    # wxq/wxk: host-packed [128, pair, chunk-in-pair, hi/lo, wq-half | x-strip0]
    wxq = [nc.dram_tensor(f"wxq{h}", [128, 2, 2, 2, DH + SUB], f8, kind="ExternalInput")
           for h in range(2)]
    wxk = [nc.dram_tensor(f"wxk{h}", [128, 2, 2, 2, DH + SUB], f8, kind="ExternalInput")
           for h in range(2)]
    wv = nc.dram_tensor("wv", [128, 8, 2, DH], f8, kind="ExternalInput")
    wo = nc.dram_tensor("wo", [DH, D], bf16, kind="ExternalInput")
    if qkv_bias:
        bq = nc.dram_tensor("bq", [DH], f32, kind="ExternalInput")
        bk = nc.dram_tensor("bk", [DH], f32, kind="ExternalInput")
        bv = nc.dram_tensor("bv", [DH], bf16, kind="ExternalInput")
    tri = nc.dram_tensor("tri", [KT, KT], bf16, kind="ExternalInput")
    maskT = None
    if not causal:
        maskT = nc.dram_tensor("maskT", [L, L], bf16, kind="ExternalInput")
    f16 = mybir.dt.float16
    out = nc.dram_tensor("out", [L, D], f16, kind="ExternalOutput")

    with tile.TileContext(nc) as tc:
        with (
            tc.tile_pool(name="consts", bufs=1) as consts,
            tc.tile_pool(name="resident", bufs=1) as resident,
            tc.tile_pool(name="xin", bufs=2) as xin,
            tc.tile_pool(name="ptile", bufs=22) as ptile,
            tc.tile_pool(name="small", bufs=7) as small,
            tc.tile_pool(name="stage", bufs=2) as stage,
            tc.tile_pool(name="mtile", bufs=4) as mtile,
            tc.tile_pool(name="bank", bufs=2, space="PSUM") as bank,
            tc.tile_pool(name="sps", bufs=2, space="PSUM") as sps,
        ):
            # ---- constants / weights ----
            # arena layout [128, pair(2), chunk-in-pair(2), hi/lo(2), DH | SUB]
            arena_q = [consts.tile([128, 2, 2, 2, DH + SUB], f8, tag=f"aq{h}", name=f"aq{h}")
                       for h in range(2)]
            arena_k = [consts.tile([128, 2, 2, 2, DH + SUB], f8, tag=f"ak{h}", name=f"ak{h}")
                       for h in range(2)]
            wv_t = consts.tile([128, 8, 2, DH], f8, tag="wv")
            wo_t = consts.tile([128, 2, D], bf16, tag="wo")
            xc = xin.tile([128, 8, 2, SUB], f8, tag="xv", name="xv_pre0")
            ssl = slice(0, SUB)
            nc.sync.dma_start(out=arena_q[0][:, 0:1], in_=wxq[0][:, 0:1])
            nc.sync.dma_start(out=arena_q[0][:, 1:2], in_=wxq[0][:, 1:2])
            # tiny consts next: the first attention needs tri soon
            bqP = bkP = bv_t = None
            if qkv_bias:
                bqP = consts.tile([128, 2], f32, tag="bqP")
                bkP = consts.tile([128, 2], f32, tag="bkP")
                nc.sync.dma_start(out=bqP, in_=bq.rearrange("(m p) -> p m", p=128))
                nc.sync.dma_start(out=bkP, in_=bk.rearrange("(m p) -> p m", p=128))
                bv_t = consts.tile([1, DH], bf16, tag="bv")
                nc.sync.dma_start(out=bv_t, in_=bv[:].unsqueeze(0))
            tri_t = consts.tile([KT, KT], bf16, tag="tri")
            nc.sync.dma_start(out=tri_t, in_=tri[:])
            nc.sync.dma_start(out=arena_q[1], in_=wxq[1][:])
            nc.sync.dma_start(out=arena_k[0], in_=wxk[0][:])
            nc.sync.dma_start(out=arena_k[1], in_=wxk[1][:])
            nc.sync.dma_start(out=wv_t, in_=wv[:])
            nc.sync.dma_start(out=xc, in_=vT[:, :, :, ssl])

            # prefetch strip 1 x into dedicated tiles so the serialized DMA
            # device works ahead while strip-0 projections run
            xa1 = xin.tile([128, 8, 2, SUB], f8, tag="xq", name="xq_pre1")
            xb1 = xin.tile([128, 8, 2, SUB], f8, tag="xk", name="xk_pre1")
            xc1 = xin.tile([128, 8, 2, SUB], f8, tag="xv", name="xv_pre1")
            x_first = [(None, None, xc), (xa1, xb1, xc1)]
            ssl1 = slice(SUB, 2 * SUB)
            nc.sync.dma_start(out=xa1, in_=qT[:, :, :, ssl1])
            nc.sync.dma_start(out=xb1, in_=kT[:, :, :, ssl1])
            nc.sync.dma_start(out=xc1, in_=vT[:, :, :, ssl1])
            # strip-2 q prefetch ahead of wo (wo isn't consumed until ~40us)
            xa2 = xin.tile([128, 8, 2, SUB], f8, tag="xq", name="xq_pre2")
            nc.sync.dma_start(out=xa2, in_=qT[:, :, :, slice(2 * SUB, 3 * SUB)])
            xb2 = xin.tile([128, 8, 2, SUB], f8, tag="xk", name="xk_pre2")
            nc.sync.dma_start(out=xb2, in_=kT[:, :, :, slice(2 * SUB, 3 * SUB)])
            xc2 = xin.tile([128, 8, 2, SUB], f8, tag="xv", name="xv_pre2")
            nc.sync.dma_start(out=xc2, in_=vT[:, :, :, slice(2 * SUB, 3 * SUB)])
            x_first.append((xa2, xb2, xc2))
            xa3 = xin.tile([128, 8, 2, SUB], f8, tag="xq", name="xq_pre3")
            nc.sync.dma_start(out=xa3, in_=qT[:, :, :, slice(3 * SUB, 4 * SUB)])
            xb3 = xin.tile([128, 8, 2, SUB], f8, tag="xk", name="xk_pre3")
            nc.sync.dma_start(out=xb3, in_=kT[:, :, :, slice(3 * SUB, 4 * SUB)])
            xc3 = xin.tile([128, 8, 2, SUB], f8, tag="xv", name="xv_pre3")
            nc.sync.dma_start(out=xc3, in_=vT[:, :, :, slice(3 * SUB, 4 * SUB)])
            x_first.append((xa3, xb3, xc3))
            nc.sync.dma_start(out=wo_t, in_=wo.rearrange("(c p) n -> p c n", p=128))
            ones_b = consts.tile([1, 128], bf16, tag="onesb")
            if qkv_bias:
                nc.vector.memset(ones_b, 1.0)

            # ---- resident activations (one tile per strip to keep
            # scheduler dependencies fine-grained) ----
            # q_s/k_s: fp8, [partition = head*32 + dk%32, half = dk//32, q]
            q_s, k_s, v_s, o_s = [], [], [], []
            for s in range(NSTRIP):
                q_tile = resident.tile([128, 2, STRIP], f8, tag=f"q{s}", name=f"q{s}")
                k_tile = resident.tile([128, 2, STRIP], f8, tag=f"k{s}", name=f"k{s}")
                # v: [partition = k within tile, ktile-in-strip, head, dk+1]
                v_tile = resident.tile([128, 4, HG, DK + 1], bf16, tag=f"v{s}", name=f"v{s}")
                nc.vector.memset(v_tile[:, :, :, DK : DK + 1], 1.0)
                o_tile = resident.tile([128, 2, STRIP], bf16, tag=f"o{s}", name=f"o{s}")
                q_s.append(q_tile); k_s.append(k_tile); v_s.append(v_tile); o_s.append(o_tile)

            # 12 DoubleRow matmuls of a hi/lo projection: per chunk-pair u,
            # (w_hi x_hi) + (w_hi x_lo) + (w_lo x_hi); lo*lo dropped.
            HL = ((0, 0), (0, 1), (1, 0))

            def proj_qk(t, arena, x_t, dst, biasP):
                for m in range(2):
                    ps = bank.tile([128, SUB], f32, tag="bank")
                    msl = slice(m * 128, (m + 1) * 128)
                    n_mm = 0
                    for u in range(4):
                        ar = arena[u // 2]
                        uu = u % 2
                        for whl, xhl in HL:
                            w_ap = ar[:, uu, 0:2, whl, msl]
                            if t == 0:
                                x_ap = ar[:, uu, 0:2, xhl, DH:DH + SUB]
                            else:
                                x_ap = x_t[:, 2 * u:2 * u + 2, xhl, :]
                            nc.tensor.matmul(ps, lhsT=w_ap, rhs=x_ap,
                                             start=(n_mm == 0), stop=(n_mm == 11),
                                             perf_mode=DR)
                            n_mm += 1
                    if qkv_bias:
                        nc.vector.tensor_scalar(
                            dst[t][:, m, :], ps, 1.0 / WS, biasP[:, m:m + 1],
                            mybir.AluOpType.mult, mybir.AluOpType.add)
                    else:
                        nc.vector.tensor_scalar_mul(dst[t][:, m, :], ps, 1.0 / WS)

            def proj_q(t):
                ctx_ = nc.named_scope(f"projq{t}"); ctx_.__enter__()
                x_q = None
                if t >= len(x_first):
                    x_q = xin.tile([128, 8, 2, SUB], f8, tag="xq")
                    sl = slice(t * SUB, (t + 1) * SUB)
                    nc.sync.dma_start(out=x_q, in_=qT[:, :, :, sl])
                elif t > 0:
                    x_q = x_first[t][0]
                proj_qk(t, arena_q, x_q, q_s, bqP)
                ctx_.__exit__(None, None, None)

            def proj_k(t):
                ctx_ = nc.named_scope(f"projk{t}"); ctx_.__enter__()
                x_k = None
                if t > 0 and (t >= len(x_first) or x_first[t][1] is None):
                    x_k = xin.tile([128, 8, 2, SUB], f8, tag="xk")
                    sl = slice(t * SUB, (t + 1) * SUB)
                    nc.sync.dma_start(out=x_k, in_=kT[:, :, :, sl])
                else:
                    x_k = x_first[t][1]
                proj_qk(t, arena_k, x_k, k_s, bkP)
                ctx_.__exit__(None, None, None)

            def proj_v(t):
                ctx_ = nc.named_scope(f"projv{t}"); ctx_.__enter__()
                if t < len(x_first) and x_first[t][2] is not None:
                    x_v = x_first[t][2]
                else:
                    x_v = xin.tile([128, 8, 2, SUB], f8, tag="xv")
                    sl = slice(t * SUB, (t + 1) * SUB)
                    nc.sync.dma_start(out=x_v, in_=vT[:, :, :, sl])
                for j in range(4):
                    ps = bank.tile([128, DH], f32, tag="bank")
                    jsl = slice(j * 128, (j + 1) * 128)
                    if qkv_bias:
                        nc.tensor.matmul(ps, lhsT=ones_b, rhs=bv_t,
                                         start=True, stop=False)
                    n_mm = 0
                    for u in range(4):
                        for xhl, whl in HL:
                            nc.tensor.matmul(
                                ps, lhsT=x_v[:, 2 * u:2 * u + 2, xhl, jsl],
                                rhs=wv_t[:, 2 * u:2 * u + 2, whl, :],
                                start=(not qkv_bias and n_mm == 0),
                                stop=(n_mm == 11),
                                perf_mode=DR)
                            n_mm += 1
                    nc.vector.tensor_scalar_mul(
                        v_s[t][:, j, :, 0:DK],
                        ps.rearrange("p (h d) -> p h d", h=HG), 1.0 / WS)
                ctx_.__exit__(None, None, None)

            def proj_kv(t):
                proj_k(t)
                proj_v(t)

            def attention_hp(s, hp, chunked=False, alt_ops=False):
                ctx_ = nc.named_scope(f"attn{s}h{hp}"); ctx_.__enter__()
                q0 = s * STRIP
                a_max = 4 * s + 3 if causal else NKT - 1

                def normalize(o_ps, i, qsl):
                    r_t = small.tile([1, STRIP], f32r, tag="recip")
                    w = qsl.stop - qsl.start
                    with nc.allow_low_precision(reason="float32r is fp32 bits"):
                        if causal:
                            nc.vector.reciprocal(r_t[:, 0:w], o_ps[i][64:65, qsl])
                        else:
                            dn = small.tile([1, STRIP], f32r, tag="denom")
                            nc.vector.tensor_scalar_max(dn[:, 0:w], o_ps[i][64:65, qsl], 1e-30)
                            nc.vector.reciprocal(r_t[:, 0:w], dn[:, 0:w])
                    bc_t = small.tile([64, STRIP], f32, tag="bc")
                    nc.gpsimd.partition_broadcast(
                        bc_t[:, 0:w], r_t[:, 0:w].bitcast(f32), channels=64)
                    nc.vector.tensor_mul(
                        o_s[s][i * 64:(i + 1) * 64, hp, qsl],
                        o_ps[i][0:64, qsl], bc_t[:, 0:w])

                # phase A (high priority): S matmuls + exp + mask, streamed
                # into pt tiles; phase B (normal priority): PV accumulation +
                # normalize.  Keeping PV out of the high-priority stream stops
                # a PV psum-pool wait from blocking the next unit's S/exp on
                # the in-order PE queue.
                pts = []
                with tc.high_priority():
                    for a in range(a_max + 1):
                        lo = max((a - 4 * s) * KT, 0) if causal else 0
                        sp = sps.tile([128, 2, STRIP], f32, tag="sps")
                        ksl = slice((a % 4) * KT, (a % 4 + 1) * KT)
                        for j in range(2):
                            h = 2 * hp + j
                            pr = slice(h * 32, (h + 1) * 32)
                            nc.tensor.matmul(
                                sp[:, j, lo:STRIP],
                                lhsT=k_s[a // 4][pr, :, ksl],
                                rhs=q_s[s][pr, :, lo:STRIP],
                                start=True, stop=True, perf_mode=DR,
                                tile_position=(h * 32, 0),
                            )
                        pt = ptile.tile([128, 2, STRIP], bf16, tag="pt")
                        nc.scalar.activation(out=pt[:, :, lo:STRIP],
                                             in_=sp[:, :, lo:STRIP],
                                             func=EXP, scale=0.125)
                        if causal and 0 <= a - 4 * s <= 3:
                            d0 = (a - 4 * s) * KT
                            eng = nc.gpsimd
                            eng.tensor_mul(
                                pt[:, :, d0:d0 + KT], pt[:, :, d0:d0 + KT],
                                tri_t.unsqueeze(1).broadcast_to((KT, 2, KT)))
                        if not causal:
                            mt = mtile.tile([128, STRIP], bf16, tag="mt")
                            nc.sync.dma_start(
                                out=mt, in_=maskT[a * KT:(a + 1) * KT, q0:q0 + STRIP])
                            for i in range(2):
                                nc.vector.tensor_mul(pt[:, i, :], pt[:, i, :], mt)
                        pts.append((a, lo, pt))
                otag = "bank" if alt_ops else "ops"
                o_ps0 = bank.tile([65, STRIP], f32, tag=otag, bufs=2)
                o_ps1 = bank.tile([65, STRIP], f32, tag=otag, bufs=2)
                o_ps = [o_ps0, o_ps1]
                for a, lo, pt in pts:
                    for i in range(2):
                        nc.tensor.matmul(o_ps[i][:, lo:STRIP],
                                         lhsT=v_s[a // 4][:, a % 4, 2 * hp + i, :],
                                         rhs=pt[:, i, lo:STRIP],
                                         start=(a == 0), stop=(a == a_max))
                    # chunked: q-chunk qc of o_ps is final once block
                    # a = 4s + qc has accumulated (later blocks only
                    # touch higher q); normalize it immediately so the
                    # Wo chunk matmuls can start before the strip ends
                    if chunked and causal and a - 4 * s in (1, 3):
                        h2 = (a - 4 * s) // 2
                        for i in range(2):
                            normalize(o_ps, i, slice(h2 * 256, (h2 + 1) * 256))
                if not (chunked and causal):
                    for i in range(2):
                        normalize(o_ps, i, slice(0, STRIP))

                ctx_.__exit__(None, None, None)

            def wo_strip(s, chunked_dma=False, act_copies=False, alt_wps=False):
                ctx_ = nc.named_scope(f"wo{s}"); ctx_.__enter__()
                st = stage.tile([128, 4, D], f16, tag="st")
                for t4 in range(4):
                    csl = slice(t4 * 128, (t4 + 1) * 128)
                    for n in range(2):
                        wps = bank.tile([128, 512], f32,
                                        tag=("ops" if alt_wps else "bank"), bufs=2)
                        nsl = slice(n * 512, (n + 1) * 512)
                        for c in range(2):
                            nc.tensor.matmul(wps, lhsT=o_s[s][:, c, csl],
                                             rhs=wo_t[:, c, nsl],
                                             start=(c == 0), stop=(c == 1))
                        if act_copies and n == 0:
                            nc.scalar.copy(out=st[:, t4, nsl], in_=wps)
                        else:
                            nc.vector.tensor_copy(st[:, t4, nsl], wps)
                    if chunked_dma:
                        r0 = s * STRIP + t4 * 128
                        nc.sync.dma_start(
                            out=out[r0:r0 + 128, :].rearrange("(t p) n -> p t n", p=128),
                            in_=st[:, t4:t4 + 1, :],
                        )
                if not chunked_dma:
                    for half in range(2):
                        r0 = s * STRIP + half * 256
                        nc.sync.dma_start(
                            out=out[r0:r0 + 256, :].rearrange("(t p) n -> p t n", p=128),
                            in_=st[:, half * 2:(half + 1) * 2, :],
                        )
                ctx_.__exit__(None, None, None)

            if causal:
                proj_q(0)
                proj_kv(0)
                proj_q(1)
                proj_kv(1)
                attention_hp(0, 0, chunked=True)
                proj_q(2)
                attention_hp(0, 1, chunked=True)
                proj_k(2)
                attention_hp(1, 0, chunked=True)
                wo_strip(0)
                proj_v(2)
                proj_q(3)
                attention_hp(1, 1, chunked=True)
                proj_k(3)
                attention_hp(2, 0, chunked=True)
                proj_v(3)
                attention_hp(2, 1, chunked=True)
                wo_strip(1, chunked_dma=True)
                attention_hp(3, 0, chunked=True)
                wo_strip(2, chunked_dma=True)
                attention_hp(3, 1, chunked=True, alt_ops=True)
                wo_strip(3, chunked_dma=True, alt_wps=True)
            else:
                # non-causal: every strip reads every K/V tile, so all
                # projections must be emitted before any attention
                for t in range(NSUB):
                    proj_q(t)
                    proj_kv(t)
                for s in range(NSTRIP):
                    attention_hp(s, 0)
                    attention_hp(s, 1)
                    wo_strip(s)

    nc.compile()
    return nc


def _get_kernel(causal: bool, qkv_bias: bool):
    key = ("attn", causal, qkv_bias)
    if key not in _CACHE:
        _CACHE[key] = _build(causal, qkv_bias)
    return _CACHE[key]


def kernel(query, key, value, mask, wq, bq, wk, bk, wv, bv, wo, bo):
    import ml_dtypes
    from concourse import bass_utils

    f32 = np.float32
    bf16 = ml_dtypes.bfloat16
    F8 = ml_dtypes.float8_e4m3

    mask_b = np.asarray(mask, dtype=bool)
    causal = bool(
        (mask_b[:, 0] == np.tril(np.ones((L, L), dtype=bool))[None]).all()
    )
    qkv_bias = bool(np.any(np.asarray(bq)) or np.any(np.asarray(bk))
                    or np.any(np.asarray(bv)))
    nc = _get_kernel(causal, qkv_bias)

    def hilo(x):
        # x: f32 -> stacked [.., 2, ..] hi/lo fp8 along a new axis=1
        hi = x.astype(F8)
        lo = (x - hi.astype(f32)).astype(F8)
        return hi, lo

    tri_np = np.triu(np.ones((KT, KT), dtype=f32)).astype(bf16)

    def x_hl(x):
        # [L, D] f32 -> ([D, 2, L] fp8 hi/lo for arena packing,
        #                [128, 8, 2, L] partition-major for the DMA tensor)
        xT = np.ascontiguousarray(np.asarray(x, f32).T)
        hi, lo = hilo(xT)
        dhl = np.ascontiguousarray(np.stack([hi, lo], axis=1))
        pcl = np.ascontiguousarray(
            dhl.reshape(8, 128, 2, L).transpose(1, 0, 2, 3))
        return dhl, pcl

    qT = [x_hl(query[b]) for b in range(B)]
    kT = [x_hl(key[b]) for b in range(B)]
    vT = [x_hl(value[b]) for b in range(B)]
    if not causal:
        maskT = [
            np.ascontiguousarray(mask_b[b, 0].T).astype(bf16) for b in range(B)
        ]

    wq = np.asarray(wq, f32)
    wk = np.asarray(wk, f32)
    wv = np.asarray(wv, f32)
    wo = np.asarray(wo, f32)
    bq = np.asarray(bq, f32)
    bk = np.asarray(bk, f32)
    bv = np.asarray(bv, f32)

    # column permutation for the dk-split-32 q_s/k_s layout:
    # psum partition p of m-chunk m holds head-dim (p//32)*64 + m*32 + p%32
    perm = np.empty(DH, np.int64)
    for m in range(2):
        for p in range(128):
            perm[m * 128 + p] = (p // 32) * 64 + m * 32 + (p % 32)

    in_maps = []
    for c in range(NCORES):
        b, g = c // TP, c % TP
        gs = slice(g * DH, (g + 1) * DH)

        def pack(w_full, xT_hl, h, permute):
            # [128, 2(pair), 2(chunk-in-pair), 2(hi/lo), DH + SUB]
            rsl = slice(512 * h, 512 * (h + 1))
            wcols = w_full[rsl, gs]
            if permute:
                wcols = wcols[:, perm]
            w_hi, w_lo = hilo(np.ascontiguousarray(wcols * WS))  # [512, DH]
            # xT_hl: [D, 2, L] fp8; strip-0 cols
            x_part = xT_hl[rsl, :, 0:SUB]                         # [512, 2, SUB]
            whl = np.stack([w_hi, w_lo], axis=1)                  # [512, 2, DH]
            arr = np.concatenate([whl, x_part], axis=2)           # [512, 2, 768]
            # rows split (pair u, chunk-in-pair e, partition p)
            return np.ascontiguousarray(
                arr.reshape(2, 2, 128, 2, DH + SUB).transpose(2, 0, 1, 3, 4))

        def pack_wv():
            w = np.ascontiguousarray(wv[:, gs] * WS)              # [D, DH]
            hi, lo = hilo(w)
            arr = np.stack([hi, lo], axis=1)                      # [D, 2, DH]
            return np.ascontiguousarray(
                arr.reshape(8, 128, 2, DH).transpose(1, 0, 2, 3))

        m = {
            "qT": qT[b][1], "kT": kT[b][1], "vT": vT[b][1],
            "wxq0": pack(wq, qT[b][0], 0, True), "wxq1": pack(wq, qT[b][0], 1, True),
            "wxk0": pack(wk, kT[b][0], 0, True), "wxk1": pack(wk, kT[b][0], 1, True),
            "wv": pack_wv(),
            "wo": np.ascontiguousarray(wo[gs, :]).astype(bf16),
            "tri": tri_np,
        }
        if qkv_bias:
            m["bq"] = np.ascontiguousarray(bq[gs][perm])
            m["bk"] = np.ascontiguousarray(bk[gs][perm])
            m["bv"] = (np.ascontiguousarray(bv[gs]) * WS).astype(bf16)
        if not causal:
            m["maskT"] = maskT[b]
        in_maps.append(m)

    res = bass_utils.run_bass_kernel_spmd(nc, in_maps, core_ids=list(range(NCORES)))

    out = np.zeros((B, L, D), f32)
    for c in range(NCORES):
        out[c // TP] += res.results[c]["out"].astype(f32)
    out += np.asarray(bo, f32)[None, None, :]
    return out


# revision 46
# speedup vs baseline: 1.0257x; 1.0257x over previous
"""Multi-head causal attention (B=2, L=2048, D=1024, H=16) on 8 trn2 cores.

Sharding: data-parallel over batch (2) x tensor-parallel over heads (4 groups
of 4 heads).  Core c handles batch c//4, heads 4*(c%4) .. 4*(c%4)+3.
Wq/Wk/Wv are column-sharded, Wo row-sharded; the TP all-reduce after Wo is
done host-side (sum of the 4 partial outputs per batch), as is the bo add.

Per-core kernel. Matmul precision scheme (all accumulation fp32 in PSUM):
  - Q/K/V projections run in fp8e4m3 with the DoubleRow perf mode (two
    128-row contraction chunks per instruction at 0.5 cycles/row).  To keep
    bf16-grade accuracy, X and W are split host-side into hi+lo fp8 pairs
    (w quantized after a x32 pre-scale so the residual stays in fp8's normal
    range; the 1/32 is folded into the PSUM->SBUF copy).  Each chunk-pair
    needs 3 DoubleRow matmuls (hi*hi, hi*lo, lo*hi; the lo*lo term is ~0.1%
    and dropped), so a projection costs 12 instrs at 0.5 cycles/row vs the
    bf16 16... i.e. 0.75x the bf16 PE time.
  - S^T = K^T Q runs in plain fp8 (Q/K quantized once at the projection
    copy; adds ~1.2e-2 rel err, the dominant error term).  The dk=64
    contraction is split 32+32 across the DoubleRow halves: q_s/k_s hold
    [partition = head*32 + dk%32, half = dk//32, q] (host permutes Wq/Wk
    columns so the projection lands directly in this layout), making each
    S block one DoubleRow matmul per head at 0.5 cycles/row - 2x the bf16
    tensor time.
  - P = exp(S/8) stays bf16 on the ACT engine (softmax WITHOUT
    max-subtraction; scores are bounded ~4 so exp is safe); PV and Wo
    matmuls stay bf16 (P has no cheap fp8 residual path and plain-fp8
    P/V/o would each cost ~2.5e-2 rel err).
  - a ones-row appended to V (lhsT [128, 65]) makes the PV matmul also emit
    softmax denominators; DVE reciprocal + gpsimd partition broadcast + one
    DVE multiply normalize O^T before the Wo matmul.
  - causal mask: S^T/exp/PV restricted to q >= k-tile start; the diagonal
    128x128 block is masked by one [128,2,128] gpsimd multiply with a
    broadcast upper-triangular 0/1 tile (supplied as input).
  - scheduling: inside each attention unit the S/exp/mask stream is emitted
    at high priority and the PV/normalize stream at normal priority, so a
    PV psum-pool wait can never block the next unit's S/exp on the in-order
    PE queue (pt depth 22 buffers the lag); o_ps gets its own 2-buffer psum
    tag so attention psum rotation is decoupled from the proj/wo tiles;
    wq/wk and the strip-0 x columns are host-packed into arena chunks so
    the first projection matmuls start after one small DMA; later x strips
    are prefetched up-front, partial outputs stored fp16, host upcasts.
"""

import numpy as np

B, L, D, H = 2, 2048, 1024, 16
DK = D // H          # 64
NCORES = 8
TP = 4               # head-group shards per batch
HG = H // TP         # 4 heads per core
DH = HG * DK         # 256 per-core head dims
STRIP = 512          # attention q-strip width
NSTRIP = L // STRIP  # 4
SUB = 512            # projection substrip width
NSUB = L // SUB      # 4
KT = 128             # key tile
NKT = L // KT        # 16
WS = 32.0            # host pre-scale on Wq/Wk/Wv so fp8 residuals stay normal

_CACHE = {}


def _build(causal: bool, qkv_bias: bool = True):
    import concourse.mybir as mybir
    import concourse.tile as tile
    from concourse import bacc

    f32 = mybir.dt.float32
    f32r = mybir.dt.float32r
    bf16 = mybir.dt.bfloat16
    f8 = mybir.dt.float8e4
    EXP = mybir.ActivationFunctionType.Exp
    DR = mybir.MatmulPerfMode.DoubleRow

    nc = bacc.Bacc("TRN2", target_bir_lowering=False)

    # x inputs as [128, chunk(8), hi/lo(2), L] fp8 (partition-major so a
    # strip slice is a 3-dim DMA: (chunk, hi/lo) merge)
    qT = nc.dram_tensor("qT", [128, 8, 2, L], f8, kind="ExternalInput")
    kT = nc.dram_tensor("kT", [128, 8, 2, L], f8, kind="ExternalInput")
    vT = nc.dram_tensor("vT", [128, 8, 2, L], f8, kind="ExternalInput")
    # wxq/wxk: host-packed [128, pair, chunk-in-pair, hi/lo, wq-half | x-strip0]
    wxq = [nc.dram_tensor(f"wxq{h}", [128, 2, 2, 2, DH + SUB], f8, kind="ExternalInput")
           for h in range(2)]
    wxk = [nc.dram_tensor(f"wxk{h}", [128, 2, 2, 2, DH + SUB], f8, kind="ExternalInput")
           for h in range(2)]
    wv = nc.dram_tensor("wv", [128, 8, 2, DH], f8, kind="ExternalInput")
    wo = nc.dram_tensor("wo", [DH, D], bf16, kind="ExternalInput")
    if qkv_bias:
        bq = nc.dram_tensor("bq", [DH], f32, kind="ExternalInput")
        bk = nc.dram_tensor("bk", [DH], f32, kind="ExternalInput")
        bv = nc.dram_tensor("bv", [DH], bf16, kind="ExternalInput")
    tri = nc.dram_tensor("tri", [KT, KT], bf16, kind="ExternalInput")
    maskT = None
    if not causal:
        maskT = nc.dram_tensor("maskT", [L, L], bf16, kind="ExternalInput")
    f16 = mybir.dt.float16
    out = nc.dram_tensor("out", [L, D], f16, kind="ExternalOutput")

    with tile.TileContext(nc) as tc:
        with (
            tc.tile_pool(name="consts", bufs=1) as consts,
            tc.tile_pool(name="resident", bufs=1) as resident,
            tc.tile_pool(name="xin", bufs=2) as xin,
            tc.tile_pool(name="ptile", bufs=22) as ptile,
            tc.tile_pool(name="small", bufs=7) as small,
            tc.tile_pool(name="stage", bufs=2) as stage,
            tc.tile_pool(name="mtile", bufs=4) as mtile,
            tc.tile_pool(name="bank", bufs=2, space="PSUM") as bank,
            tc.tile_pool(name="sps", bufs=2, space="PSUM") as sps,
        ):
            # ---- constants / weights ----
            # arena layout [128, pair(2), chunk-in-pair(2), hi/lo(2), DH | SUB]
            arena_q = [consts.tile([128, 2, 2, 2, DH + SUB], f8, tag=f"aq{h}", name=f"aq{h}")
                       for h in range(2)]
            arena_k = [consts.tile([128, 2, 2, 2, DH + SUB], f8, tag=f"ak{h}", name=f"ak{h}")
                       for h in range(2)]
            wv_t = consts.tile([128, 8, 2, DH], f8, tag="wv")
            wo_t = consts.tile([128, 2, D], bf16, tag="wo")
            xc = xin.tile([128, 8, 2, SUB], f8, tag="xv", name="xv_pre0")
            ssl = slice(0, SUB)
            nc.sync.dma_start(out=arena_q[0][:, 0:1], in_=wxq[0][:, 0:1])
            nc.sync.dma_start(out=arena_q[0][:, 1:2], in_=wxq[0][:, 1:2])
            # tiny consts next: the first attention needs tri soon
            bqP = bkP = bv_t = None
            if qkv_bias:
                bqP = consts.tile([128, 2], f32, tag="bqP")
                bkP = consts.tile([128, 2], f32, tag="bkP")
                nc.sync.dma_start(out=bqP, in_=bq.rearrange("(m p) -> p m", p=128))
                nc.sync.dma_start(out=bkP, in_=bk.rearrange("(m p) -> p m", p=128))
                bv_t = consts.tile([1, DH], bf16, tag="bv")
                nc.sync.dma_start(out=bv_t, in_=bv[:].unsqueeze(0))
            tri_t = consts.tile([KT, KT], bf16, tag="tri")
            nc.sync.dma_start(out=tri_t, in_=tri[:])
            nc.sync.dma_start(out=arena_q[1], in_=wxq[1][:])
            nc.sync.dma_start(out=arena_k[0], in_=wxk[0][:])
            nc.sync.dma_start(out=arena_k[1], in_=wxk[1][:])
            nc.sync.dma_start(out=wv_t, in_=wv[:])
            nc.sync.dma_start(out=xc, in_=vT[:, :, :, ssl])

            # prefetch strip 1 x into dedicated tiles so the serialized DMA
            # device works ahead while strip-0 projections run
            xa1 = xin.tile([128, 8, 2, SUB], f8, tag="xq", name="xq_pre1")
            xb1 = xin.tile([128, 8, 2, SUB], f8, tag="xk", name="xk_pre1")
            xc1 = xin.tile([128, 8, 2, SUB], f8, tag="xv", name="xv_pre1")
            x_first = [(None, None, xc), (xa1, xb1, xc1)]
            ssl1 = slice(SUB, 2 * SUB)
            nc.sync.dma_start(out=xa1, in_=qT[:, :, :, ssl1])
            nc.sync.dma_start(out=xb1, in_=kT[:, :, :, ssl1])
            nc.sync.dma_start(out=xc1, in_=vT[:, :, :, ssl1])
            # strip-2 q prefetch ahead of wo (wo isn't consumed until ~40us)
            xa2 = xin.tile([128, 8, 2, SUB], f8, tag="xq", name="xq_pre2")
            nc.sync.dma_start(out=xa2, in_=qT[:, :, :, slice(2 * SUB, 3 * SUB)])
            xb2 = xin.tile([128, 8, 2, SUB], f8, tag="xk", name="xk_pre2")
            nc.sync.dma_start(out=xb2, in_=kT[:, :, :, slice(2 * SUB, 3 * SUB)])
            xc2 = xin.tile([128, 8, 2, SUB], f8, tag="xv", name="xv_pre2")
            nc.sync.dma_start(out=xc2, in_=vT[:, :, :, slice(2 * SUB, 3 * SUB)])
            x_first.append((xa2, xb2, xc2))
            xa3 = xin.tile([128, 8, 2, SUB], f8, tag="xq", name="xq_pre3")
            nc.sync.dma_start(out=xa3, in_=qT[:, :, :, slice(3 * SUB, 4 * SUB)])
            xb3 = xin.tile([128, 8, 2, SUB], f8, tag="xk", name="xk_pre3")
            nc.sync.dma_start(out=xb3, in_=kT[:, :, :, slice(3 * SUB, 4 * SUB)])
            xc3 = xin.tile([128, 8, 2, SUB], f8, tag="xv", name="xv_pre3")
            nc.sync.dma_start(out=xc3, in_=vT[:, :, :, slice(3 * SUB, 4 * SUB)])
            x_first.append((xa3, xb3, xc3))
            nc.sync.dma_start(out=wo_t, in_=wo.rearrange("(c p) n -> p c n", p=128))
            ones_b = consts.tile([1, 128], bf16, tag="onesb")
            if qkv_bias:
                nc.vector.memset(ones_b, 1.0)

            # ---- resident activations (one tile per strip to keep
            # scheduler dependencies fine-grained) ----
            # q_s/k_s: fp8, [partition = head*32 + dk%32, half = dk//32, q]
            q_s, k_s, v_s, o_s = [], [], [], []
            for s in range(NSTRIP):
                q_tile = resident.tile([128, 2, STRIP], f8, tag=f"q{s}", name=f"q{s}")
                k_tile = resident.tile([128, 2, STRIP], f8, tag=f"k{s}", name=f"k{s}")
                # v: [partition = k within tile, ktile-in-strip, head, dk+1]
                v_tile = resident.tile([128, 4, HG, DK + 1], bf16, tag=f"v{s}", name=f"v{s}")
                nc.vector.memset(v_tile[:, :, :, DK : DK + 1], 1.0)
                o_tile = resident.tile([128, 2, STRIP], bf16, tag=f"o{s}", name=f"o{s}")
                q_s.append(q_tile); k_s.append(k_tile); v_s.append(v_tile); o_s.append(o_tile)

            # 12 DoubleRow matmuls of a hi/lo projection: per chunk-pair u,
            # (w_hi x_hi) + (w_hi x_lo) + (w_lo x_hi); lo*lo dropped.
            HL = ((0, 0), (0, 1), (1, 0))

            def proj_qk(t, arena, x_t, dst, biasP):
                for m in range(2):
                    ps = bank.tile([128, SUB], f32, tag="bank")
                    msl = slice(m * 128, (m + 1) * 128)
                    n_mm = 0
                    for u in range(4):
                        ar = arena[u // 2]
                        uu = u % 2
                        for whl, xhl in HL:
                            w_ap = ar[:, uu, 0:2, whl, msl]
                            if t == 0:
                                x_ap = ar[:, uu, 0:2, xhl, DH:DH + SUB]
                            else:
                                x_ap = x_t[:, 2 * u:2 * u + 2, xhl, :]
                            nc.tensor.matmul(ps, lhsT=w_ap, rhs=x_ap,
                                             start=(n_mm == 0), stop=(n_mm == 11),
                                             perf_mode=DR)
                            n_mm += 1
                    if qkv_bias:
                        nc.vector.tensor_scalar(
                            dst[t][:, m, :], ps, 1.0 / WS, biasP[:, m:m + 1],
                            mybir.AluOpType.mult, mybir.AluOpType.add)
                    else:
                        nc.vector.tensor_scalar_mul(dst[t][:, m, :], ps, 1.0 / WS)

            def proj_q(t):
                ctx_ = nc.named_scope(f"projq{t}"); ctx_.__enter__()
                x_q = None
                if t >= len(x_first):
                    x_q = xin.tile([128, 8, 2, SUB], f8, tag="xq")
                    sl = slice(t * SUB, (t + 1) * SUB)
                    nc.sync.dma_start(out=x_q, in_=qT[:, :, :, sl])
                elif t > 0:
                    x_q = x_first[t][0]
                proj_qk(t, arena_q, x_q, q_s, bqP)
                ctx_.__exit__(None, None, None)

            def proj_k(t):
                ctx_ = nc.named_scope(f"projk{t}"); ctx_.__enter__()
                x_k = None
                if t > 0 and (t >= len(x_first) or x_first[t][1] is None):
                    x_k = xin.tile([128, 8, 2, SUB], f8, tag="xk")
                    sl = slice(t * SUB, (t + 1) * SUB)
                    nc.sync.dma_start(out=x_k, in_=kT[:, :, :, sl])
                else:
                    x_k = x_first[t][1]
                proj_qk(t, arena_k, x_k, k_s, bkP)
                ctx_.__exit__(None, None, None)

            def proj_v(t):
                ctx_ = nc.named_scope(f"projv{t}"); ctx_.__enter__()
                if t < len(x_first) and x_first[t][2] is not None:
                    x_v = x_first[t][2]
                else:
                    x_v = xin.tile([128, 8, 2, SUB], f8, tag="xv")
                    sl = slice(t * SUB, (t + 1) * SUB)
                    nc.sync.dma_start(out=x_v, in_=vT[:, :, :, sl])
                for j in range(4):
                    ps = bank.tile([128, DH], f32, tag="bank")
                    jsl = slice(j * 128, (j + 1) * 128)
                    if qkv_bias:
                        nc.tensor.matmul(ps, lhsT=ones_b, rhs=bv_t,
                                         start=True, stop=False)
                    n_mm = 0
                    for u in range(4):
                        for xhl, whl in HL:
                            nc.tensor.matmul(
                                ps, lhsT=x_v[:, 2 * u:2 * u + 2, xhl, jsl],
                                rhs=wv_t[:, 2 * u:2 * u + 2, whl, :],
                                start=(not qkv_bias and n_mm == 0),
                                stop=(n_mm == 11),
                                perf_mode=DR)
                            n_mm += 1
                    nc.vector.tensor_scalar_mul(
                        v_s[t][:, j, :, 0:DK],
                        ps.rearrange("p (h d) -> p h d", h=HG), 1.0 / WS)
                ctx_.__exit__(None, None, None)

            def proj_kv(t):
                proj_k(t)
                proj_v(t)

            def attention_hp(s, hp, chunked=False, alt_ops=False):
                ctx_ = nc.named_scope(f"attn{s}h{hp}"); ctx_.__enter__()
                q0 = s * STRIP
                a_max = 4 * s + 3 if causal else NKT - 1

                def normalize(o_ps, i, qsl):
                    r_t = small.tile([1, STRIP], f32r, tag="recip")
                    w = qsl.stop - qsl.start
                    with nc.allow_low_precision(reason="float32r is fp32 bits"):
                        if causal:
                            nc.vector.reciprocal(r_t[:, 0:w], o_ps[i][64:65, qsl])
                        else:
                            dn = small.tile([1, STRIP], f32r, tag="denom")
                            nc.vector.tensor_scalar_max(dn[:, 0:w], o_ps[i][64:65, qsl], 1e-30)
                            nc.vector.reciprocal(r_t[:, 0:w], dn[:, 0:w])
                    bc_t = small.tile([64, STRIP], f32, tag="bc")
                    nc.gpsimd.partition_broadcast(
                        bc_t[:, 0:w], r_t[:, 0:w].bitcast(f32), channels=64)
                    nc.vector.tensor_mul(
                        o_s[s][i * 64:(i + 1) * 64, hp, qsl],
                        o_ps[i][0:64, qsl], bc_t[:, 0:w])

                # phase A (high priority): S matmuls + exp + mask, streamed
                # into pt tiles; phase B (normal priority): PV accumulation +
                # normalize.  Keeping PV out of the high-priority stream stops
                # a PV psum-pool wait from blocking the next unit's S/exp on
                # the in-order PE queue.
                pts = []
                with tc.high_priority():
                    for a in range(a_max + 1):
                        lo = max((a - 4 * s) * KT, 0) if causal else 0
                        sp = sps.tile([128, 2, STRIP], f32, tag="sps")
                        ksl = slice((a % 4) * KT, (a % 4 + 1) * KT)
                        for j in range(2):
                            h = 2 * hp + j
                            pr = slice(h * 32, (h + 1) * 32)
                            nc.tensor.matmul(
                                sp[:, j, lo:STRIP],
                                lhsT=k_s[a // 4][pr, :, ksl],
                                rhs=q_s[s][pr, :, lo:STRIP],
                                start=True, stop=True, perf_mode=DR,
                                tile_position=(h * 32, 0),
                            )
                        pt = ptile.tile([128, 2, STRIP], bf16, tag="pt")
                        nc.scalar.activation(out=pt[:, :, lo:STRIP],
                                             in_=sp[:, :, lo:STRIP],
                                             func=EXP, scale=0.125)
                        if causal and 0 <= a - 4 * s <= 3:
                            d0 = (a - 4 * s) * KT
                            eng = nc.gpsimd
                            eng.tensor_mul(
                                pt[:, :, d0:d0 + KT], pt[:, :, d0:d0 + KT],
                                tri_t.unsqueeze(1).broadcast_to((KT, 2, KT)))
                        if not causal:
                            mt = mtile.tile([128, STRIP], bf16, tag="mt")
                            nc.sync.dma_start(
                                out=mt, in_=maskT[a * KT:(a + 1) * KT, q0:q0 + STRIP])
                            for i in range(2):
                                nc.vector.tensor_mul(pt[:, i, :], pt[:, i, :], mt)
                        pts.append((a, lo, pt))
                otag = "bank" if alt_ops else "ops"
                o_ps0 = bank.tile([65, STRIP], f32, tag=otag, bufs=2)
                o_ps1 = bank.tile([65, STRIP], f32, tag=otag, bufs=2)
                o_ps = [o_ps0, o_ps1]
                for a, lo, pt in pts:
                    for i in range(2):
                        nc.tensor.matmul(o_ps[i][:, lo:STRIP],
                                         lhsT=v_s[a // 4][:, a % 4, 2 * hp + i, :],
                                         rhs=pt[:, i, lo:STRIP],
                                         start=(a == 0), stop=(a == a_max))
                    # chunked: q-chunk qc of o_ps is final once block
                    # a = 4s + qc has accumulated (later blocks only
                    # touch higher q); normalize it immediately so the
                    # Wo chunk matmuls can start before the strip ends
                    if chunked and causal and a - 4 * s in (1, 3):
                        h2 = (a - 4 * s) // 2
                        for i in range(2):
                            normalize(o_ps, i, slice(h2 * 256, (h2 + 1) * 256))
                if not (chunked and causal):
                    for i in range(2):
                        normalize(o_ps, i, slice(0, STRIP))

                ctx_.__exit__(None, None, None)

            def wo_strip(s, chunked_dma=False, act_copies=False, alt_wps=False):
                ctx_ = nc.named_scope(f"wo{s}"); ctx_.__enter__()
                st = stage.tile([128, 4, D], f16, tag="st")
                for t4 in range(4):
                    csl = slice(t4 * 128, (t4 + 1) * 128)
                    for n in range(2):
                        wps = bank.tile([128, 512], f32,
                                        tag=("ops" if alt_wps else "bank"), bufs=2)
                        nsl = slice(n * 512, (n + 1) * 512)
                        for c in range(2):
                            nc.tensor.matmul(wps, lhsT=o_s[s][:, c, csl],
                                             rhs=wo_t[:, c, nsl],
                                             start=(c == 0), stop=(c == 1))
                        if act_copies and n == 0:
                            nc.scalar.copy(out=st[:, t4, nsl], in_=wps)
                        else:
                            nc.vector.tensor_copy(st[:, t4, nsl], wps)
                    if chunked_dma:
                        r0 = s * STRIP + t4 * 128
                        nc.sync.dma_start(
                            out=out[r0:r0 + 128, :].rearrange("(t p) n -> p t n", p=128),
                            in_=st[:, t4:t4 + 1, :],
                        )
                if not chunked_dma:
                    for half in range(2):
                        r0 = s * STRIP + half * 256
                        nc.sync.dma_start(
                            out=out[r0:r0 + 256, :].rearrange("(t p) n -> p t n", p=128),
                            in_=st[:, half * 2:(half + 1) * 2, :],
                        )
                ctx_.__exit__(None, None, None)

            if causal:
                proj_q(0)
                proj_kv(0)
                proj_q(1)
                proj_kv(1)
                attention_hp(0, 0, chunked=True)
                proj_q(2)
                attention_hp(0, 1, chunked=True)
                proj_k(2)
                attention_hp(1, 0, chunked=True)
                wo_strip(0)
                proj_v(2)
                proj_q(3)
                attention_hp(1, 1, chunked=True)
                proj_k(3)
                attention_hp(2, 0, chunked=True)
                proj_v(3)
                attention_hp(2, 1, chunked=True)
                wo_strip(1, chunked_dma=True)
                attention_hp(3, 0, chunked=True)
                wo_strip(2, chunked_dma=True)
                attention_hp(3, 1, chunked=True)
                wo_strip(3, chunked_dma=True, act_copies=True)
            else:
                # non-causal: every strip reads every K/V tile, so all
                # projections must be emitted before any attention
                for t in range(NSUB):
                    proj_q(t)
                    proj_kv(t)
                for s in range(NSTRIP):
                    attention_hp(s, 0)
                    attention_hp(s, 1)
                    wo_strip(s)

    nc.compile()
    return nc


def _get_kernel(causal: bool, qkv_bias: bool):
    key = ("attn", causal, qkv_bias)
    if key not in _CACHE:
        _CACHE[key] = _build(causal, qkv_bias)
    return _CACHE[key]


def kernel(query, key, value, mask, wq, bq, wk, bk, wv, bv, wo, bo):
    import ml_dtypes
    from concourse import bass_utils

    f32 = np.float32
    bf16 = ml_dtypes.bfloat16
    F8 = ml_dtypes.float8_e4m3

    mask_b = np.asarray(mask, dtype=bool)
    causal = bool(
        (mask_b[:, 0] == np.tril(np.ones((L, L), dtype=bool))[None]).all()
    )
    qkv_bias = bool(np.any(np.asarray(bq)) or np.any(np.asarray(bk))
                    or np.any(np.asarray(bv)))
    nc = _get_kernel(causal, qkv_bias)

    def hilo(x):
        # x: f32 -> stacked [.., 2, ..] hi/lo fp8 along a new axis=1
        hi = x.astype(F8)
        lo = (x - hi.astype(f32)).astype(F8)
        return hi, lo

    tri_np = np.triu(np.ones((KT, KT), dtype=f32)).astype(bf16)

    def x_hl(x):
        # [L, D] f32 -> ([D, 2, L] fp8 hi/lo for arena packing,
        #                [128, 8, 2, L] partition-major for the DMA tensor)
        xT = np.ascontiguousarray(np.asarray(x, f32).T)
        hi, lo = hilo(xT)
        dhl = np.ascontiguousarray(np.stack([hi, lo], axis=1))
        pcl = np.ascontiguousarray(
            dhl.reshape(8, 128, 2, L).transpose(1, 0, 2, 3))
        return dhl, pcl

    qT = [x_hl(query[b]) for b in range(B)]
    kT = [x_hl(key[b]) for b in range(B)]
    vT = [x_hl(value[b]) for b in range(B)]
    if not causal:
        maskT = [
            np.ascontiguousarray(mask_b[b, 0].T).astype(bf16) for b in range(B)
        ]

    wq = np.asarray(wq, f32)
    wk = np.asarray(wk, f32)
    wv = np.asarray(wv, f32)
    wo = np.asarray(wo, f32)
    bq = np.asarray(bq, f32)
    bk = np.asarray(bk, f32)
    bv = np.asarray(bv, f32)

    # column permutation for the dk-split-32 q_s/k_s layout:
    # psum partition p of m-chunk m holds head-dim (p//32)*64 + m*32 + p%32
    perm = np.empty(DH, np.int64)
    for m in range(2):
        for p in range(128):
            perm[m * 128 + p] = (p // 32) * 64 + m * 32 + (p % 32)

    in_maps = []
    for c in range(NCORES):
        b, g = c // TP, c % TP
        gs = slice(g * DH, (g + 1) * DH)

        def pack(w_full, xT_hl, h, permute):
            # [128, 2(pair), 2(chunk-in-pair), 2(hi/lo), DH + SUB]
            rsl = slice(512 * h, 512 * (h + 1))
            wcols = w_full[rsl, gs]
            if permute:
                wcols = wcols[:, perm]
            w_hi, w_lo = hilo(np.ascontiguousarray(wcols * WS))  # [512, DH]
            # xT_hl: [D, 2, L] fp8; strip-0 cols
            x_part = xT_hl[rsl, :, 0:SUB]                         # [512, 2, SUB]
            whl = np.stack([w_hi, w_lo], axis=1)                  # [512, 2, DH]
            arr = np.concatenate([whl, x_part], axis=2)           # [512, 2, 768]
            # rows split (pair u, chunk-in-pair e, partition p)
            return np.ascontiguousarray(
                arr.reshape(2, 2, 128, 2, DH + SUB).transpose(2, 0, 1, 3, 4))

        def pack_wv():
            w = np.ascontiguousarray(wv[:, gs] * WS)              # [D, DH]
            hi, lo = hilo(w)
            arr = np.stack([hi, lo], axis=1)                      # [D, 2, DH]
            return np.ascontiguousarray(
                arr.reshape(8, 128, 2, DH).transpose(1, 0, 2, 3))

        m = {
            "qT": qT[b][1], "kT": kT[b][1], "vT": vT[b][1],
            "wxq0": pack(wq, qT[b][0], 0, True), "wxq1": pack(wq, qT[b][0], 1, True),
            "wxk0": pack(wk, kT[b][0], 0, True), "wxk1": pack(wk, kT[b][0], 1, True),
            "wv": pack_wv(),
            "wo": np.ascontiguousarray(wo[gs, :]).astype(bf16),
            "tri": tri_np,
        }
        if qkv_bias:
            m["bq"] = np.ascontiguousarray(bq[gs][perm])
            m["bk"] = np.ascontiguousarray(bk[gs][perm])
            m["bv"] = (np.ascontiguousarray(bv[gs]) * WS).astype(bf16)
        if not causal:
            m["maskT"] = maskT[b]
        in_maps.append(m)

    res = bass_utils.run_bass_kernel_spmd(nc, in_maps, core_ids=list(range(NCORES)))

    out = np.zeros((B, L, D), f32)
    for c in range(NCORES):
        out[c // TP] += res.results[c]["out"].astype(f32)
    out += np.asarray(bo, f32)[None, None, :]
    return out
